# revision 15
# baseline (speedup 1.0000x reference)
"""BallLoss Trainium2 kernel v4 (8-core data-parallel SPMD).

loss = sum_{i,j} relu(d_i - d_ij),  d_ij = ||e_i - c_j||, d_i = d_{i,label_i}

Per-core, rows sharded along N (8192/core), centers replicated. Identity:
    sum_j relu(d_i - d_ij) = C*d_i - sum_j min(d_ij, d_i)

Pipeline per [128, 2048] row-tile (one full row block x all centers):
  - PE:   ps[i,j] = (c2_j - 2*e_i.c_j)/16 via augmented fp16 matmul
          (lhsT = [e;1]^T K=65, rhs = [-2c/16; c2/16]); fp16 keeps the
          cross term at ~2^-11 and c2/16 fits one fp16 row (no hi/lo
          split); the /16 scaling is undone by the ACT scale.
  - ACT:  dist = sqrt(16*ps + e2) -> fp16 SBUF (single fused PSUM
          evacuation; bias = e2 per partition, scale = 16).
  - DVE:  V-tile: ts min(dist, d_i) in-place (4x), tt acc += dist (2x)
          into a shared fp16 accumulator [128, 2048] (only the global
          sum matters, so rows from different tiles share cells).
  - ACT:  VR-tile (optional, n_va>0): min+reduce on ACT instead via
          relu(-dist + d_i) with accum_out. Measured OFF by default:
          each extra ACT op per tile adds a ~1.7us pipeline bubble
          (the 2-deep PSUM ping-pong can't absorb the hiccup).
  - DVE:  D-tile (optional poly path): DVE evacuates PSUM itself:
          m = (ps + e2/16) min (d2/16) -> fp16 (1x), u = a2*m + a1
          (ts 4x), t = u*m (tt 2x), acc += t (tt 2x); sqrt(16m) ~
          a2*m^2 + a1*m + a0 fitted on the data distribution; the
          a0*2048-per-row constant is folded into the final scalar.
  - chat: built off the DVE critical path: -2c/16 rows on gpsimd from
          a scalar-queue craw load; c2 in natural layout ([128,16]
          square+reduce) then moved into the [1,2048] rhs row via a
          32x32 xbar transpose DMA.
  - d_i:  exact from per-tile indirect-DMA gathers of c[label]:
          diff = e - c_lab (gpsimd, fp16 out), d2 = sum diff^2 (DVE
          fp16 square at 2x + reduce), d_i = sqrt(d2) (ACT, per group).
  - final: loss = C*(sum_i d_i - sum_{VR rows} d_i) - sum(acc)
           + sum(maccA) - a0*2048*128*nD, reduced on-chip.

Scheduling: 8-tile groups (loads + gathers + per-row precompute + main
tiles) with explicit ordering deps keeping the gather-gated d2 chain
behind the previous group's main DVE ops.

Host: shards inputs, provides e / e^T layouts (fp16 casts) and int32
labels (layout prep only), sums the 8 per-core scalars.
"""

from contextlib import ExitStack

import numpy as np

import concourse.bass as bass
import concourse.tile as tile
from concourse import bacc, mybir
from concourse.bass_utils import run_bass_kernel_spmd

F32 = mybir.dt.float32
F16 = mybir.dt.float16
I32 = mybir.dt.int32
AF = mybir.ActivationFunctionType
OP = mybir.AluOpType
AX = mybir.AxisListType

N, C, D = 65536, 2048, 64
NCORES = 8
NS = N // NCORES  # 8192 rows per core
P = 128           # partitions
T = NS // P       # 64 row-tiles per core
FD = 512          # fp32 psum bank free dim
NB = C // FD      # 4 matmuls per row-tile
G = 8             # row-tiles per precompute group
NG = T // G       # 8 groups
CT = C // P       # 16 center-chunks of 128 for the natural-layout c2

MM_DT = F16
KA = D + 1
SCL = 16.0        # matmul output scale divisor

# quadratic fit of sqrt(16*m) over the min(s_ij, s_i)/16 data
# distribution; a0 is folded into the final scalar.
PA0 = 4.22324667
PA1 = 1.05483169
PA2 = -0.02105464

N_VA = 0          # VR tiles, at uniform stride T//N_VA
N_D = 0


def _tile_types(n_va, n_d):
    types = [0] * T
    if n_va:
        stride = T // n_va
        for k in range(n_va):
            types[stride // 2 + k * stride] = 1
    if n_d:
        free = [t for t in range(T) if types[t] == 0]
        for k in range(n_d):
            types[free[int((k + 0.5) * len(free) / n_d)]] = 2
    return types


def _body(tc, out, eT, enat, labT, cT, cnat, n_va=N_VA, n_d=N_D):
    nc = tc.nc
    types = _tile_types(n_va, n_d)
    nD = types.count(2)
    with ExitStack() as ctx:
        const = ctx.enter_context(tc.tile_pool(name="const", bufs=1))

        eTa = const.tile([KA, NS], MM_DT)   # [65, 8192] rows 0..63 = e^T, 64 = 1
        chat = const.tile([KA, C], MM_DT)   # [65, 2048] 0..63 = -2c^T/16, 64 = c2/16
        craw = const.tile([D, C], F32)      # raw c^T
        csq = const.tile([D, C], F16)       # chat^2 = c^2/64, for the c2 colsum
        ones64 = const.tile([D, 1], MM_DT)
        ensb = const.tile([P, T * D], F16)  # [128, 4096] e natural fp16, tile-major
        clab = const.tile([P, T * D], F32)  # gathered centers per row (f32)
        diff = const.tile([P, T * D], F16)  # e - c_lab, then squared in place
        scrh = const.tile([P, T * D], F16)  # e^2 scratch
        labsb = const.tile([P, T], I32)
        e2 = const.tile([P, T], F16)
        e2s = const.tile([P, T], F32)       # e2/16 (D-path only)
        d2 = const.tile([P, T], F16)
        d2s = const.tile([P, T], F32)       # d2/16 (D-path only)
        dall = const.tile([P, T], F32)
        maccA = const.tile([P, T], F32)     # VR-tile ACT accum row sums
        paired = (n_va == 0 and n_d == 0)
        accw = 2 * C if paired else C
        acc = const.tile([P, accw], F16)    # shared elementwise accumulator
        rowacc = const.tile([P, 1], F32)
        dsum = const.tile([P, 1], F32)
        dsumv = const.tile([P, 1], F32)
        rowA = const.tile([P, 1], F32)
        rowfin = const.tile([P, 1], F32)
        onesr = const.tile([P, 1], F32)
        outsb = const.tile([1, 1], F32)

        # labels first: the gpsimd gather stream is gated only on this DMA
        nc.sync.dma_start(labsb[:], labT)
        nc.vector.memset(onesr[:], 1.0)
        nc.vector.memset(ones64[:], 1.0)
        # warm the ACT table with a Sqrt op so the single table load picks
        # the sqrt set (which also serves Identity/Relu); otherwise the
        # chat-row Identity ops load a non-sqrt table and the first main
        # sqrt pays a 1.3us re-load on the critical path
        nc.scalar.activation(dsum[:], onesr[:], AF.Sqrt)

        mm_ctx = tc.tile_pool(name="mm", bufs=2, space="PSUM")
        mm_pool = mm_ctx.__enter__()

        # chat build, pipelined per 512-col bank chunk. craw rides the
        # (idle-at-startup) scalar engine's DMA queue; csq is chat^2 at
        # 2x (c^2/64); the c2/16 row lands via an ACT copy with scale=4.
        # The colsum scratch borrows an mm-pool slot so the main-loop
        # psum allocation isn't serialized behind a pool release.
        c2ps_full = mm_pool.tile([P, C], F32, name="ps", tag="ps")
        c2ps = c2ps_full[0:1, :]
        for k in range(NB):
            sl = slice(k * FD, (k + 1) * FD)
            nc.scalar.dma_start(craw[:, sl], cT[:, sl])
            nc.vector.tensor_scalar_mul(chat[0:D, sl], craw[:, sl], -2.0 / SCL)
            c2cp = nc.vector.tensor_mul(csq[:, sl], chat[0:D, sl], chat[0:D, sl])
            if k == 1:
                c2cp_early = c2cp
            nc.tensor.matmul(
                c2ps[:, sl], lhsT=ones64[:], rhs=csq[:, sl],
                start=True, stop=True,
            )
            # Identity (not Copy): keeps the ACT in the sqrt table set so
            # the first main sqrt doesn't pay a 1.3us table re-load
            nc.scalar.activation(chat[D:D + 1, sl], c2ps[:, sl], AF.Identity,
                                 scale=SCL / 4.0)

        # big memsets after the chat chain so they don't delay it
        nc.vector.memset(acc[:], 0.0)
        if n_va:
            nc.vector.memset(maccA[:], 0.0)

        dist_pool = ctx.enter_context(
            tc.tile_pool(name="dist", bufs=4 if paired else 8))
        from concourse.tile import add_dep_helper

        dve_anchor = {}
        prev_gpsub = None

        def anchor_for(t):
            while t >= 0 and t not in dve_anchor:
                t -= 1
            return dve_anchor.get(t)

        # first group split into 2-tile mini-groups so tile 0's d2 chain
        # waits on only 2 gathers and the first sqrt fires ~7us earlier
        bounds = [(0, 2), (2, 4), (4, G)] + [
            (g * G, (g + 1) * G) for g in range(1, NG)
        ]
        for gi, (ts_, te) in enumerate(bounds):
            cs, ce = ts_ * P, te * P
            fs, fe = ts_ * D, te * D
            nc.sync.dma_start(eTa[:, cs:ce], eT[:, cs:ce])
            nc.sync.dma_start(
                ensb[:, fs:fe].rearrange("p (t d) -> p t d", d=D),
                enat[cs:ce, :].rearrange("(t p) d -> p t d", p=P),
            )
            first_gather = None
            for t in range(ts_, te):
                g_inst = nc.gpsimd.indirect_dma_start(
                    out=clab[:, t * D:(t + 1) * D],
                    out_offset=None,
                    in_=cnat,
                    in_offset=bass.IndirectOffsetOnAxis(ap=labsb[:, t:t + 1], axis=0),
                )
                if first_gather is None:
                    first_gather = g_inst
            if gi in (1, 2) and prev_gpsub is not None:
                # don't let the scheduler hoist these gathers ahead of the
                # previous mini-group's sub: the sub unblocks the d2 chain
                # that gates the very first tiles
                add_dep_helper(first_gather.ins, prev_gpsub.ins, sync=False,
                               reason="first gathers after prior sub")
            # per-row e2 (fp16 square at 2x, reduce to f32)
            nc.vector.tensor_mul(scrh[:, fs:fe], ensb[:, fs:fe], ensb[:, fs:fe])
            with nc.allow_low_precision(reason="fp16 e2: d err ~1e-3 abs, random per row"):
                nc.vector.tensor_reduce(
                    e2[:, ts_:te], scrh[:, fs:fe].rearrange("p (t d) -> p t d", d=D),
                    axis=AX.X, op=OP.add,
                )
            # d2 chain: gather-gated
            prev_gpsub = nc.gpsimd.tensor_sub(
                diff[:, fs:fe], ensb[:, fs:fe], clab[:, fs:fe]
            )
            sub_i = nc.vector.tensor_mul(
                diff[:, fs:fe], diff[:, fs:fe], diff[:, fs:fe]
            )
            if gi >= 3:
                # keep the gather-gated d2 chain BEHIND the previous group's
                # main DVE ops in the scheduled stream (the scheduler's DMA
                # model thinks indirect gathers are cheap; at runtime they'd
                # stall the whole in-order DVE stream if hoisted early)
                a = anchor_for(ts_ - 3)
                if a is not None:
                    add_dep_helper(sub_i.ins, a.ins, sync=False,
                                   reason="hold d2 chain behind prior group")
            elif gi in (1, 2):
                a = anchor_for(ts_ - 2)
                if a is not None:
                    add_dep_helper(sub_i.ins, a.ins, sync=False,
                                   reason="hold d2 chain behind prior group")
            else:
                # group 0 only needs to sit behind an EARLY chat op, not the
                # whole build: chunk 1's csq keeps the DVE stream clean while
                # letting the first d2 complete ~5us sooner
                add_dep_helper(sub_i.ins, c2cp_early.ins, sync=False,
                               reason="hold g0 d2 chain behind chat chunk 1")
            with nc.allow_low_precision(reason="fp16 d2: d err ~1e-2 abs, random per row"):
                nc.vector.tensor_reduce(
                    d2[:, ts_:te], diff[:, fs:fe].rearrange("p (t d) -> p t d", d=D),
                    axis=AX.X, op=OP.add,
                )
            nc.scalar.activation(dall[:, ts_:te], d2[:, ts_:te], AF.Sqrt)
            if nD:
                nc.vector.tensor_scalar_mul(e2s[:, ts_:te], e2[:, ts_:te], 1.0 / SCL)
                nc.vector.tensor_scalar_mul(d2s[:, ts_:te], d2[:, ts_:te], 1.0 / SCL)

            # main tiles of this group
            for t in range(ts_, te):
                ps = mm_pool.tile([P, C], F32, name="ps")
                lhsT = eTa[:, t * P:(t + 1) * P]
                for k in range(NB):
                    nc.tensor.matmul(
                        ps[:, k * FD:(k + 1) * FD],
                        lhsT=lhsT,
                        rhs=chat[:, k * FD:(k + 1) * FD],
                        start=True, stop=True,
                    )
                ty = types[t]
                if paired:
                    if t % 2 == 0:
                        dpair = dist_pool.tile([P, 2 * C], F16, name="dist")
                        half = dpair[:, 0:C]
                    else:
                        half = dpair[:, C:2 * C]
                    nc.scalar.activation(
                        half, ps[:], AF.Sqrt,
                        bias=e2[:, t:t + 1], scale=SCL,
                    )
                    mi = nc.vector.tensor_scalar(
                        out=half, in0=half,
                        scalar1=dall[:, t:t + 1], scalar2=None,
                        op0=OP.min,
                    )
                    dve_anchor[t] = mi
                    if t % 2 == 1:
                        nc.vector.tensor_add(acc[:], acc[:], dpair[:])
                    continue
                if ty == 2:
                    # D-tile: DVE evacuates PSUM, quadratic sqrt approx
                    m = dist_pool.tile([P, C], F16, name="dist")
                    mi = nc.vector.tensor_scalar(
                        out=m[:], in0=ps[:],
                        scalar1=e2s[:, t:t + 1], scalar2=d2s[:, t:t + 1],
                        op0=OP.add, op1=OP.min,
                    )
                    u = dist_pool.tile([P, C], F16, name="dist")
                    nc.vector.tensor_scalar(
                        out=u[:], in0=m[:], scalar1=PA2, scalar2=PA1,
                        op0=OP.mult, op1=OP.add,
                    )
                    nc.vector.tensor_mul(u[:], u[:], m[:])
                    nc.vector.tensor_add(acc[:], acc[:], u[:])
                    dve_anchor[t] = mi
                else:
                    dist = dist_pool.tile([P, C], F16, name="dist")
                    nc.scalar.activation(
                        dist[:], ps[:], AF.Sqrt,
                        bias=e2[:, t:t + 1], scale=SCL,
                    )
                    if ty == 1:
                        # VR-tile: the whole min+reduce runs on ACT:
                        # sum_j relu(d_i - d_ij) via scale=-1, bias=d_i
                        nc.scalar.activation(
                            dist[:], dist[:], AF.Relu,
                            bias=dall[:, t:t + 1], scale=-1.0,
                            accum_out=maccA[:, t:t + 1],
                        )
                    else:
                        mi = nc.vector.tensor_scalar(
                            out=dist[:], in0=dist[:],
                            scalar1=dall[:, t:t + 1], scalar2=None,
                            op0=OP.min,
                        )
                        nc.vector.tensor_add(acc[:], acc[:], dist[:])
                        dve_anchor[t] = mi

        mm_ctx.__exit__(None, None, None)

        # loss = C*(sum_i d_i - sum_{VR} d_i) - sum(acc) + sum(maccA)
        #        - a0*2048*128*nD
        nc.vector.tensor_reduce(rowacc[:], acc[:], axis=AX.X, op=OP.add)
        nc.vector.tensor_reduce(dsum[:], dall[:], axis=AX.X, op=OP.add)
        if n_va:
            nc.vector.tensor_reduce(rowA[:], maccA[:], axis=AX.X, op=OP.add)
        if n_va:
            stride = T // n_va
            dallv = dall[:].rearrange("p (g s) -> p s g", s=stride)
            nc.vector.tensor_reduce(
                dsumv[:], dallv[:, stride // 2:stride // 2 + 1, :],
                axis=AX.X, op=OP.add,
            )
            nc.vector.tensor_sub(dsum[:], dsum[:], dsumv[:])
        nc.vector.scalar_tensor_tensor(
            out=rowfin[:], in0=dsum[:], scalar=float(C), op0=OP.mult,
            in1=rowacc[:], op1=OP.subtract,
        )
        if n_va:
            nc.vector.tensor_add(rowfin[:], rowfin[:], rowA[:])
        with tc.tile_pool(name="fin", bufs=1, space="PSUM") as finp:
            fin = finp.tile([1, 1], F32)
            nc.tensor.matmul(fin[:], lhsT=rowfin[:], rhs=onesr[:], start=True, stop=True)
            nc.scalar.activation(
                outsb[:], fin[:], AF.Identity,
                bias=-PA0 * float(C) * float(P) * float(nD),
            )
        nc.sync.dma_start(out, outsb[:])


_NC_CACHE = {}


def build_nc(n_va=N_VA, n_d=N_D):
    key = (n_va, n_d)
    if key in _NC_CACHE:
        return _NC_CACHE[key]
    nc = bacc.Bacc(
        "TRN2", target_bir_lowering=False, debug=False, enable_asserts=False
    )
    eT = nc.dram_tensor("eT", [KA, NS], MM_DT, kind="ExternalInput").ap()
    enat = nc.dram_tensor("enat", [NS, D], F16, kind="ExternalInput").ap()
    labT = nc.dram_tensor("labT", [P, T], I32, kind="ExternalInput").ap()
    cT = nc.dram_tensor("cT", [D, C], F32, kind="ExternalInput").ap()
    cnat = nc.dram_tensor("cnat", [C, D], F32, kind="ExternalInput").ap()
    out = nc.dram_tensor("out", [1, 1], F32, kind="ExternalOutput").ap()
    with tile.TileContext(nc) as tc:
        _body(tc, out, eT, enat, labT, cT, cnat, n_va=n_va, n_d=n_d)
    nc.compile()
    _NC_CACHE[key] = nc
    return nc


def make_in_maps(embeddings, centers, labels):
    e = np.ascontiguousarray(np.asarray(embeddings, dtype=np.float32))
    c = np.ascontiguousarray(np.asarray(centers, dtype=np.float32))
    lab = np.asarray(labels).astype(np.int32)
    assert e.shape == (N, D) and c.shape == (C, D) and lab.shape == (N,)
    cT = np.ascontiguousarray(c.T)
    in_maps = []
    for core in range(NCORES):
        es = e[core * NS:(core + 1) * NS]
        ls = lab[core * NS:(core + 1) * NS]
        eTa = np.ones((KA, NS), np.float32)
        eTa[0:D] = es.T
        eTa = eTa.astype(np.float16)
        in_maps.append({
            "eT": eTa,
            "enat": np.ascontiguousarray(es).astype(np.float16),
            "labT": np.ascontiguousarray(ls.reshape(T, P).T),
            "cT": cT,
            "cnat": c,
        })
    return in_maps


def run(embeddings, centers, labels, n_va=N_VA, n_d=N_D, **kw):
    nc = build_nc(n_va, n_d)
    in_maps = make_in_maps(embeddings, centers, labels)
    res = run_bass_kernel_spmd(nc, in_maps, core_ids=list(range(NCORES)), **kw)
    total = float(sum(float(r["out"][0, 0]) for r in res.results))
    return np.float32(total), res


def kernel(embeddings, centers, labels):
    val, _ = run(embeddings, centers, labels)
    return val


# revision 16
# speedup vs baseline: 1.0221x; 1.0221x over previous
"""BallLoss Trainium2 kernel v4 (8-core data-parallel SPMD).

loss = sum_{i,j} relu(d_i - d_ij),  d_ij = ||e_i - c_j||, d_i = d_{i,label_i}

Per-core, rows sharded along N (8192/core), centers replicated. Identity:
    sum_j relu(d_i - d_ij) = C*d_i - sum_j min(d_ij, d_i)

Pipeline per [128, 2048] row-tile (one full row block x all centers):
  - PE:   ps[i,j] = (c2_j - 2*e_i.c_j)/16 via augmented fp16 matmul
          (lhsT = [e;1]^T K=65, rhs = [-2c/16; c2/16]); fp16 keeps the
          cross term at ~2^-11 and c2/16 fits one fp16 row (no hi/lo
          split); the /16 scaling is undone by the ACT scale.
  - ACT:  dist = sqrt(16*ps + e2) -> fp16 SBUF (single fused PSUM
          evacuation; bias = e2 per partition, scale = 16).
  - DVE:  V-tile: ts min(dist, d_i) in-place (4x), tt acc += dist (2x)
          into a shared fp16 accumulator [128, 2048] (only the global
          sum matters, so rows from different tiles share cells).
  - ACT:  VR-tile (optional, n_va>0): min+reduce on ACT instead via
          relu(-dist + d_i) with accum_out. Measured OFF by default:
          each extra ACT op per tile adds a ~1.7us pipeline bubble
          (the 2-deep PSUM ping-pong can't absorb the hiccup).
  - DVE:  D-tile (optional poly path): DVE evacuates PSUM itself:
          m = (ps + e2/16) min (d2/16) -> fp16 (1x), u = a2*m + a1
          (ts 4x), t = u*m (tt 2x), acc += t (tt 2x); sqrt(16m) ~
          a2*m^2 + a1*m + a0 fitted on the data distribution; the
          a0*2048-per-row constant is folded into the final scalar.
  - chat: built off the DVE critical path: -2c/16 rows on gpsimd from
          a scalar-queue craw load; c2 in natural layout ([128,16]
          square+reduce) then moved into the [1,2048] rhs row via a
          32x32 xbar transpose DMA.
  - d_i:  exact from per-tile indirect-DMA gathers of c[label]:
          diff = e - c_lab (gpsimd, fp16 out), d2 = sum diff^2 (DVE
          fp16 square at 2x + reduce), d_i = sqrt(d2) (ACT, per group).
  - final: loss = C*(sum_i d_i - sum_{VR rows} d_i) - sum(acc)
           + sum(maccA) - a0*2048*128*nD, reduced on-chip.

Scheduling: 8-tile groups (loads + gathers + per-row precompute + main
tiles) with explicit ordering deps keeping the gather-gated d2 chain
behind the previous group's main DVE ops.

Host: shards inputs, provides e / e^T layouts (fp16 casts) and int32
labels (layout prep only), sums the 8 per-core scalars.
"""

from contextlib import ExitStack

import numpy as np

import concourse.bass as bass
import concourse.tile as tile
from concourse import bacc, mybir
from concourse.bass_utils import run_bass_kernel_spmd

F32 = mybir.dt.float32
F16 = mybir.dt.float16
I32 = mybir.dt.int32
AF = mybir.ActivationFunctionType
OP = mybir.AluOpType
AX = mybir.AxisListType

N, C, D = 65536, 2048, 64
NCORES = 8
NS = N // NCORES  # 8192 rows per core
P = 128           # partitions
T = NS // P       # 64 row-tiles per core
FD = 512          # fp32 psum bank free dim
NB = C // FD      # 4 matmuls per row-tile
G = 8             # row-tiles per precompute group
NG = T // G       # 8 groups
CT = C // P       # 16 center-chunks of 128 for the natural-layout c2

MM_DT = F16
KA = D + 1
SCL = 16.0        # matmul output scale divisor

# quadratic fit of sqrt(16*m) over the min(s_ij, s_i)/16 data
# distribution; a0 is folded into the final scalar.
PA0 = 4.22324667
PA1 = 1.05483169
PA2 = -0.02105464

N_VA = 0          # VR tiles, at uniform stride T//N_VA
N_D = 0


def _tile_types(n_va, n_d):
    types = [0] * T
    if n_va:
        stride = T // n_va
        for k in range(n_va):
            types[stride // 2 + k * stride] = 1
    if n_d:
        free = [t for t in range(T) if types[t] == 0]
        for k in range(n_d):
            types[free[int((k + 0.5) * len(free) / n_d)]] = 2
    return types


def _body(tc, out, eT, enat, labT, cT, cnat, n_va=N_VA, n_d=N_D):
    nc = tc.nc
    types = _tile_types(n_va, n_d)
    nD = types.count(2)
    with ExitStack() as ctx:
        const = ctx.enter_context(tc.tile_pool(name="const", bufs=1))

        eTa = const.tile([KA, NS], MM_DT)   # [65, 8192] rows 0..63 = e^T, 64 = 1
        chat = const.tile([KA, C], MM_DT)   # [65, 2048] 0..63 = -2c^T/16, 64 = c2/16
        craw = const.tile([D, C], F32)      # raw c^T
        csq = const.tile([D, C], F16)       # chat^2 = c^2/64, for the c2 colsum
        ones64 = const.tile([D, 1], MM_DT)
        ensb = const.tile([P, T * D], F16)  # [128, 4096] e natural fp16, tile-major
        clab = const.tile([P, T * D], F32)  # gathered centers per row (f32)
        diff = const.tile([P, T * D], F16)  # e - c_lab, then squared in place
        scrh = const.tile([P, T * D], F16)  # e^2 scratch
        labsb = const.tile([P, T], I32)
        e2 = const.tile([P, T], F16)
        e2s = const.tile([P, T], F32)       # e2/16 (D-path only)
        d2 = const.tile([P, T], F16)
        d2s = const.tile([P, T], F32)       # d2/16 (D-path only)
        dall = const.tile([P, T], F32)
        maccA = const.tile([P, T], F32)     # VR-tile ACT accum row sums
        acc = const.tile([P, C], F16)       # shared elementwise accumulator
        rowacc = const.tile([P, 1], F32)
        dsum = const.tile([P, 1], F32)
        dsumv = const.tile([P, 1], F32)
        rowA = const.tile([P, 1], F32)
        rowfin = const.tile([P, 1], F32)
        onesr = const.tile([P, 1], F32)
        outsb = const.tile([1, 1], F32)

        # labels first: the gpsimd gather stream is gated only on this DMA
        nc.sync.dma_start(labsb[:], labT)
        nc.vector.memset(onesr[:], 1.0)
        nc.vector.memset(ones64[:], 1.0)
        # warm the ACT table with a Sqrt op so the single table load picks
        # the sqrt set (which also serves Identity/Relu); otherwise the
        # chat-row Identity ops load a non-sqrt table and the first main
        # sqrt pays a 1.3us re-load on the critical path
        nc.scalar.activation(dsum[:], onesr[:], AF.Sqrt)

        mm_ctx = tc.tile_pool(name="mm", bufs=2, space="PSUM")
        mm_pool = mm_ctx.__enter__()

        # chat build, pipelined per 512-col bank chunk. craw rides the
        # (idle-at-startup) scalar engine's DMA queue; csq is chat^2 at
        # 2x (c^2/64); the c2/16 row lands via an ACT copy with scale=4.
        # The colsum scratch borrows an mm-pool slot so the main-loop
        # psum allocation isn't serialized behind a pool release.
        c2ps_full = mm_pool.tile([P, C], F32, name="ps", tag="ps")
        c2ps = c2ps_full[0:1, :]
        for k in range(NB):
            sl = slice(k * FD, (k + 1) * FD)
            nc.scalar.dma_start(craw[:, sl], cT[:, sl])
            nc.vector.tensor_scalar_mul(chat[0:D, sl], craw[:, sl], -2.0 / SCL)
            c2cp = nc.vector.tensor_mul(csq[:, sl], chat[0:D, sl], chat[0:D, sl])
            if k == 1:
                c2cp_early = c2cp
            nc.tensor.matmul(
                c2ps[:, sl], lhsT=ones64[:], rhs=csq[:, sl],
                start=True, stop=True,
            )
            # Identity (not Copy): keeps the ACT in the sqrt table set so
            # the first main sqrt doesn't pay a 1.3us table re-load
            nc.scalar.activation(chat[D:D + 1, sl], c2ps[:, sl], AF.Identity,
                                 scale=SCL / 4.0)

        # big memsets after the chat chain so they don't delay it
        nc.vector.memset(acc[:], 0.0)
        if n_va:
            nc.vector.memset(maccA[:], 0.0)

        dist_pool = ctx.enter_context(tc.tile_pool(name="dist", bufs=8))
        from concourse.tile import add_dep_helper

        dve_anchor = {}
        prev_gpsub = None

        def anchor_for(t):
            while t >= 0 and t not in dve_anchor:
                t -= 1
            return dve_anchor.get(t)

        # first group split into 2-tile mini-groups so tile 0's d2 chain
        # waits on only 2 gathers and the first sqrt fires ~7us earlier
        bounds = [(0, 2), (2, 4), (4, G)] + [
            (g * G, (g + 1) * G) for g in range(1, NG)
        ]
        for gi, (ts_, te) in enumerate(bounds):
            cs, ce = ts_ * P, te * P
            fs, fe = ts_ * D, te * D
            nc.sync.dma_start(eTa[:, cs:ce], eT[:, cs:ce])
            nc.sync.dma_start(
                ensb[:, fs:fe].rearrange("p (t d) -> p t d", d=D),
                enat[cs:ce, :].rearrange("(t p) d -> p t d", p=P),
            )
            first_gather = None
            for t in range(ts_, te):
                g_inst = nc.gpsimd.indirect_dma_start(
                    out=clab[:, t * D:(t + 1) * D],
                    out_offset=None,
                    in_=cnat,
                    in_offset=bass.IndirectOffsetOnAxis(ap=labsb[:, t:t + 1], axis=0),
                )
                if first_gather is None:
                    first_gather = g_inst
            if gi in (1, 2) and prev_gpsub is not None:
                # don't let the scheduler hoist these gathers ahead of the
                # previous mini-group's sub: the sub unblocks the d2 chain
                # that gates the very first tiles
                add_dep_helper(first_gather.ins, prev_gpsub.ins, sync=False,
                               reason="first gathers after prior sub")
            # per-row e2 (fp16 square at 2x, reduce to f32)
            nc.vector.tensor_mul(scrh[:, fs:fe], ensb[:, fs:fe], ensb[:, fs:fe])
            with nc.allow_low_precision(reason="fp16 e2: d err ~1e-3 abs, random per row"):
                nc.vector.tensor_reduce(
                    e2[:, ts_:te], scrh[:, fs:fe].rearrange("p (t d) -> p t d", d=D),
                    axis=AX.X, op=OP.add,
                )
            # d2 chain: gather-gated
            prev_gpsub = nc.gpsimd.tensor_sub(
                diff[:, fs:fe], ensb[:, fs:fe], clab[:, fs:fe]
            )
            sub_i = nc.vector.tensor_mul(
                diff[:, fs:fe], diff[:, fs:fe], diff[:, fs:fe]
            )
            if gi >= 3:
                # keep the gather-gated d2 chain BEHIND the previous group's
                # main DVE ops in the scheduled stream (the scheduler's DMA
                # model thinks indirect gathers are cheap; at runtime they'd
                # stall the whole in-order DVE stream if hoisted early)
                a = anchor_for(ts_ - 3)
                if a is not None:
                    add_dep_helper(sub_i.ins, a.ins, sync=False,
                                   reason="hold d2 chain behind prior group")
            elif gi in (1, 2):
                a = anchor_for(ts_ - 2)
                if a is not None:
                    add_dep_helper(sub_i.ins, a.ins, sync=False,
                                   reason="hold d2 chain behind prior group")
            else:
                # group 0 only needs to sit behind an EARLY chat op, not the
                # whole build: chunk 1's csq keeps the DVE stream clean while
                # letting the first d2 complete ~5us sooner
                add_dep_helper(sub_i.ins, c2cp_early.ins, sync=False,
                               reason="hold g0 d2 chain behind chat chunk 1")
            with nc.allow_low_precision(reason="fp16 d2: d err ~1e-2 abs, random per row"):
                nc.vector.tensor_reduce(
                    d2[:, ts_:te], diff[:, fs:fe].rearrange("p (t d) -> p t d", d=D),
                    axis=AX.X, op=OP.add,
                )
            nc.scalar.activation(dall[:, ts_:te], d2[:, ts_:te], AF.Sqrt)
            if nD:
                nc.vector.tensor_scalar_mul(e2s[:, ts_:te], e2[:, ts_:te], 1.0 / SCL)
                nc.vector.tensor_scalar_mul(d2s[:, ts_:te], d2[:, ts_:te], 1.0 / SCL)

            # main tiles of this group
            for t in range(ts_, te):
                ps = mm_pool.tile([P, C], F32, name="ps")
                lhsT = eTa[:, t * P:(t + 1) * P]
                for k in range(NB):
                    nc.tensor.matmul(
                        ps[:, k * FD:(k + 1) * FD],
                        lhsT=lhsT,
                        rhs=chat[:, k * FD:(k + 1) * FD],
                        start=True, stop=True,
                    )
                ty = types[t]
                if ty == 2:
                    # D-tile: DVE evacuates PSUM, quadratic sqrt approx
                    m = dist_pool.tile([P, C], F16, name="dist")
                    mi = nc.vector.tensor_scalar(
                        out=m[:], in0=ps[:],
                        scalar1=e2s[:, t:t + 1], scalar2=d2s[:, t:t + 1],
                        op0=OP.add, op1=OP.min,
                    )
                    u = dist_pool.tile([P, C], F16, name="dist")
                    nc.vector.tensor_scalar(
                        out=u[:], in0=m[:], scalar1=PA2, scalar2=PA1,
                        op0=OP.mult, op1=OP.add,
                    )
                    nc.vector.tensor_mul(u[:], u[:], m[:])
                    nc.vector.tensor_add(acc[:], acc[:], u[:])
                    dve_anchor[t] = mi
                else:
                    dist = dist_pool.tile([P, C], F16, name="dist")
                    nc.scalar.activation(
                        dist[:], ps[:], AF.Sqrt,
                        bias=e2[:, t:t + 1], scale=SCL,
                    )
                    if ty == 1:
                        # VR-tile: the whole min+reduce runs on ACT:
                        # sum_j relu(d_i - d_ij) via scale=-1, bias=d_i
                        nc.scalar.activation(
                            dist[:], dist[:], AF.Relu,
                            bias=dall[:, t:t + 1], scale=-1.0,
                            accum_out=maccA[:, t:t + 1],
                        )
                    else:
                        mi = nc.vector.tensor_scalar(
                            out=dist[:], in0=dist[:],
                            scalar1=dall[:, t:t + 1], scalar2=None,
                            op0=OP.min,
                        )
                        nc.vector.tensor_add(acc[:], acc[:], dist[:])
                        dve_anchor[t] = mi

        mm_ctx.__exit__(None, None, None)

        # loss = C*(sum_i d_i - sum_{VR} d_i) - sum(acc) + sum(maccA)
        #        - a0*2048*128*nD
        nc.vector.tensor_reduce(rowacc[:], acc[:], axis=AX.X, op=OP.add)
        nc.vector.tensor_reduce(dsum[:], dall[:], axis=AX.X, op=OP.add)
        if n_va:
            nc.vector.tensor_reduce(rowA[:], maccA[:], axis=AX.X, op=OP.add)
        if n_va:
            stride = T // n_va
            dallv = dall[:].rearrange("p (g s) -> p s g", s=stride)
            nc.vector.tensor_reduce(
                dsumv[:], dallv[:, stride // 2:stride // 2 + 1, :],
                axis=AX.X, op=OP.add,
            )
            nc.vector.tensor_sub(dsum[:], dsum[:], dsumv[:])
        nc.vector.scalar_tensor_tensor(
            out=rowfin[:], in0=dsum[:], scalar=float(C), op0=OP.mult,
            in1=rowacc[:], op1=OP.subtract,
        )
        if n_va:
            nc.vector.tensor_add(rowfin[:], rowfin[:], rowA[:])
        with tc.tile_pool(name="fin", bufs=1, space="PSUM") as finp:
            fin = finp.tile([1, 1], F32)
            nc.tensor.matmul(fin[:], lhsT=rowfin[:], rhs=onesr[:], start=True, stop=True)
            nc.scalar.activation(
                outsb[:], fin[:], AF.Identity,
                bias=-PA0 * float(C) * float(P) * float(nD),
            )
        nc.sync.dma_start(out, outsb[:])


_NC_CACHE = {}


def build_nc(n_va=N_VA, n_d=N_D):
    key = (n_va, n_d)
    if key in _NC_CACHE:
        return _NC_CACHE[key]
    nc = bacc.Bacc(
        "TRN2", target_bir_lowering=False, debug=False, enable_asserts=False
    )
    eT = nc.dram_tensor("eT", [KA, NS], MM_DT, kind="ExternalInput").ap()
    enat = nc.dram_tensor("enat", [NS, D], F16, kind="ExternalInput").ap()
    labT = nc.dram_tensor("labT", [P, T], I32, kind="ExternalInput").ap()
    cT = nc.dram_tensor("cT", [D, C], F32, kind="ExternalInput").ap()
    cnat = nc.dram_tensor("cnat", [C, D], F32, kind="ExternalInput").ap()
    out = nc.dram_tensor("out", [1, 1], F32, kind="ExternalOutput").ap()
    with tile.TileContext(nc) as tc:
        _body(tc, out, eT, enat, labT, cT, cnat, n_va=n_va, n_d=n_d)
    nc.compile()
    _NC_CACHE[key] = nc
    return nc


def make_in_maps(embeddings, centers, labels):
    e = np.ascontiguousarray(np.asarray(embeddings, dtype=np.float32))
    c = np.ascontiguousarray(np.asarray(centers, dtype=np.float32))
    lab = np.asarray(labels).astype(np.int32)
    assert e.shape == (N, D) and c.shape == (C, D) and lab.shape == (N,)
    cT = np.ascontiguousarray(c.T)
    in_maps = []
    for core in range(NCORES):
        es = e[core * NS:(core + 1) * NS]
        ls = lab[core * NS:(core + 1) * NS]
        eTa = np.ones((KA, NS), np.float32)
        eTa[0:D] = es.T
        eTa = eTa.astype(np.float16)
        in_maps.append({
            "eT": eTa,
            "enat": np.ascontiguousarray(es).astype(np.float16),
            "labT": np.ascontiguousarray(ls.reshape(T, P).T),
            "cT": cT,
            "cnat": c,
        })
    return in_maps


def run(embeddings, centers, labels, n_va=N_VA, n_d=N_D, **kw):
    nc = build_nc(n_va, n_d)
    in_maps = make_in_maps(embeddings, centers, labels)
    res = run_bass_kernel_spmd(nc, in_maps, core_ids=list(range(NCORES)), **kw)
    total = float(sum(float(r["out"][0, 0]) for r in res.results))
    return np.float32(total), res


def kernel(embeddings, centers, labels):
    val, _ = run(embeddings, centers, labels)
    return val


# revision 17
# speedup vs baseline: 1.0245x; 1.0024x over previous
"""BallLoss Trainium2 kernel v4 (8-core data-parallel SPMD).

loss = sum_{i,j} relu(d_i - d_ij),  d_ij = ||e_i - c_j||, d_i = d_{i,label_i}

Per-core, rows sharded along N (8192/core), centers replicated. Identity:
    sum_j relu(d_i - d_ij) = C*d_i - sum_j min(d_ij, d_i)

Pipeline per [128, 2048] row-tile (one full row block x all centers):
  - PE:   ps[i,j] = (c2_j - 2*e_i.c_j)/16 via augmented fp16 matmul
          (lhsT = [e;1]^T K=65, rhs = [-2c/16; c2/16]); fp16 keeps the
          cross term at ~2^-11 and c2/16 fits one fp16 row (no hi/lo
          split); the /16 scaling is undone by the ACT scale.
  - ACT:  dist = sqrt(16*ps + e2) -> fp16 SBUF (single fused PSUM
          evacuation; bias = e2 per partition, scale = 16).
  - DVE:  V-tile: ts min(dist, d_i) in-place (4x), tt acc += dist (2x)
          into a shared fp16 accumulator [128, 2048] (only the global
          sum matters, so rows from different tiles share cells).
  - ACT:  VR-tile (optional, n_va>0): min+reduce on ACT instead via
          relu(-dist + d_i) with accum_out. Measured OFF by default:
          each extra ACT op per tile adds a ~1.7us pipeline bubble
          (the 2-deep PSUM ping-pong can't absorb the hiccup).
  - DVE:  D-tile (optional poly path): DVE evacuates PSUM itself:
          m = (ps + e2/16) min (d2/16) -> fp16 (1x), u = a2*m + a1
          (ts 4x), t = u*m (tt 2x), acc += t (tt 2x); sqrt(16m) ~
          a2*m^2 + a1*m + a0 fitted on the data distribution; the
          a0*2048-per-row constant is folded into the final scalar.
  - chat: built off the DVE critical path: -2c/16 rows on gpsimd from
          a scalar-queue craw load; c2 in natural layout ([128,16]
          square+reduce) then moved into the [1,2048] rhs row via a
          32x32 xbar transpose DMA.
  - d_i:  exact from per-tile indirect-DMA gathers of c[label]:
          diff = e - c_lab (gpsimd, fp16 out), d2 = sum diff^2 (DVE
          fp16 square at 2x + reduce), d_i = sqrt(d2) (ACT, per group).
  - final: loss = C*(sum_i d_i - sum_{VR rows} d_i) - sum(acc)
           + sum(maccA) - a0*2048*128*nD, reduced on-chip.

Scheduling: 8-tile groups (loads + gathers + per-row precompute + main
tiles) with explicit ordering deps keeping the gather-gated d2 chain
behind the previous group's main DVE ops.

Host: shards inputs, provides e / e^T layouts (fp16 casts) and int32
labels (layout prep only), sums the 8 per-core scalars.
"""

from contextlib import ExitStack

import numpy as np

import concourse.bass as bass
import concourse.tile as tile
from concourse import bacc, mybir
from concourse.bass_utils import run_bass_kernel_spmd

F32 = mybir.dt.float32
F16 = mybir.dt.float16
I32 = mybir.dt.int32
AF = mybir.ActivationFunctionType
OP = mybir.AluOpType
AX = mybir.AxisListType

N, C, D = 65536, 2048, 64
NCORES = 8
NS = N // NCORES  # 8192 rows per core
P = 128           # partitions
T = NS // P       # 64 row-tiles per core
FD = 512          # fp32 psum bank free dim
NB = C // FD      # 4 matmuls per row-tile
G = 8             # row-tiles per precompute group
NG = T // G       # 8 groups
CT = C // P       # 16 center-chunks of 128 for the natural-layout c2

MM_DT = F16
KA = D + 1
SCL = 16.0        # matmul output scale divisor

# quadratic fit of sqrt(16*m) over the min(s_ij, s_i)/16 data
# distribution; a0 is folded into the final scalar.
PA0 = 4.22324667
PA1 = 1.05483169
PA2 = -0.02105464

N_VA = 0          # VR tiles, at uniform stride T//N_VA
N_D = 0


def _tile_types(n_va, n_d):
    types = [0] * T
    if n_va:
        stride = T // n_va
        for k in range(n_va):
            types[stride // 2 + k * stride] = 1
    if n_d:
        free = [t for t in range(T) if types[t] == 0]
        for k in range(n_d):
            types[free[int((k + 0.5) * len(free) / n_d)]] = 2
    return types


def _body(tc, out, eT, enat, labT, cT, cnat, n_va=N_VA, n_d=N_D):
    nc = tc.nc
    types = _tile_types(n_va, n_d)
    nD = types.count(2)
    with ExitStack() as ctx:
        const = ctx.enter_context(tc.tile_pool(name="const", bufs=1))

        eTa = const.tile([KA, NS], MM_DT)   # [65, 8192] rows 0..63 = e^T, 64 = 1
        chat = const.tile([KA, C], MM_DT)   # [65, 2048] 0..63 = -2c^T/16, 64 = c2/16
        craw = const.tile([D, C], F32)      # raw c^T
        csq = const.tile([D, C], F16)       # chat^2 = c^2/64, for the c2 colsum
        ones64 = const.tile([D, 1], MM_DT)
        ensb = const.tile([P, T * D], F16)  # [128, 4096] e natural fp16, tile-major
        clab = const.tile([P, T * D], F32)  # gathered centers per row (f32)
        diff = const.tile([P, T * D], F16)  # e - c_lab, then squared in place
        scrh = const.tile([P, T * D], F16)  # e^2 scratch
        labsb = const.tile([P, T], I32)
        e2 = const.tile([P, T], F16)
        e2s = const.tile([P, T], F32)       # e2/16 (D-path only)
        d2 = const.tile([P, T], F16)
        d2s = const.tile([P, T], F32)       # d2/16 (D-path only)
        dall = const.tile([P, T], F32)
        maccA = const.tile([P, T], F32)     # VR-tile ACT accum row sums
        acc = const.tile([P, C], F16)       # shared elementwise accumulator
        rowacc = const.tile([P, 1], F32)
        dsum = const.tile([P, 1], F32)
        dsumv = const.tile([P, 1], F32)
        rowA = const.tile([P, 1], F32)
        rowfin = const.tile([P, 1], F32)
        onesr = const.tile([P, 1], F32)
        outsb = const.tile([1, 1], F32)

        # labels first: the gpsimd gather stream is gated only on this DMA
        nc.sync.dma_start(labsb[:], labT)
        nc.vector.memset(onesr[:], 1.0)
        nc.vector.memset(ones64[:], 1.0)
        # warm the ACT table with a Sqrt op so the single table load picks
        # the sqrt set (which also serves Identity/Relu); otherwise the
        # chat-row Identity ops load a non-sqrt table and the first main
        # sqrt pays a 1.3us re-load on the critical path
        nc.scalar.activation(dsum[:], onesr[:], AF.Sqrt)

        mm_ctx = tc.tile_pool(name="mm", bufs=2, space="PSUM")
        mm_pool = mm_ctx.__enter__()

        # chat build, pipelined per 512-col bank chunk. craw rides the
        # (idle-at-startup) scalar engine's DMA queue; csq is chat^2 at
        # 2x (c^2/64); the c2/16 row lands via an ACT copy with scale=4.
        # The colsum scratch borrows an mm-pool slot so the main-loop
        # psum allocation isn't serialized behind a pool release.
        c2ps_full = mm_pool.tile([P, C], F32, name="ps", tag="ps")
        c2ps = c2ps_full[0:1, :]
        for k in range(NB):
            sl = slice(k * FD, (k + 1) * FD)
            nc.scalar.dma_start(craw[:, sl], cT[:, sl])
            nc.vector.tensor_scalar_mul(chat[0:D, sl], craw[:, sl], -2.0 / SCL)
            c2cp = nc.vector.tensor_mul(csq[:, sl], chat[0:D, sl], chat[0:D, sl])
            if k == 1:
                c2cp_early = c2cp
            nc.tensor.matmul(
                c2ps[:, sl], lhsT=ones64[:], rhs=csq[:, sl],
                start=True, stop=True,
            )
            # Identity (not Copy): keeps the ACT in the sqrt table set so
            # the first main sqrt doesn't pay a 1.3us table re-load
            nc.scalar.activation(chat[D:D + 1, sl], c2ps[:, sl], AF.Identity,
                                 scale=SCL / 4.0)

        # big memsets after the chat chain so they don't delay it
        nc.vector.memset(acc[:], 0.0)
        if n_va:
            nc.vector.memset(maccA[:], 0.0)

        dist_pool = ctx.enter_context(tc.tile_pool(name="dist", bufs=8))
        from concourse.tile import add_dep_helper

        dve_anchor = {}
        prev_gpsub = None

        def anchor_for(t):
            while t >= 0 and t not in dve_anchor:
                t -= 1
            return dve_anchor.get(t)

        # first group split in half so the first tiles only wait on 4 gathers
        bounds = [(0, G // 2), (G // 2, G)] + [
            (g * G, (g + 1) * G) for g in range(1, NG)
        ]
        for gi, (ts_, te) in enumerate(bounds):
            cs, ce = ts_ * P, te * P
            fs, fe = ts_ * D, te * D
            nc.sync.dma_start(eTa[:, cs:ce], eT[:, cs:ce])
            nc.sync.dma_start(
                ensb[:, fs:fe].rearrange("p (t d) -> p t d", d=D),
                enat[cs:ce, :].rearrange("(t p) d -> p t d", p=P),
            )
            for t in range(ts_, te):
                nc.gpsimd.indirect_dma_start(
                    out=clab[:, t * D:(t + 1) * D],
                    out_offset=None,
                    in_=cnat,
                    in_offset=bass.IndirectOffsetOnAxis(ap=labsb[:, t:t + 1], axis=0),
                )
            # per-row e2 (fp16 square at 2x, reduce to f32)
            nc.vector.tensor_mul(scrh[:, fs:fe], ensb[:, fs:fe], ensb[:, fs:fe])
            with nc.allow_low_precision(reason="fp16 e2: d err ~1e-3 abs, random per row"):
                nc.vector.tensor_reduce(
                    e2[:, ts_:te], scrh[:, fs:fe].rearrange("p (t d) -> p t d", d=D),
                    axis=AX.X, op=OP.add,
                )
            # d2 chain: gather-gated
            prev_gpsub = nc.gpsimd.tensor_sub(
                diff[:, fs:fe], ensb[:, fs:fe], clab[:, fs:fe]
            )
            sub_i = nc.vector.tensor_mul(
                diff[:, fs:fe], diff[:, fs:fe], diff[:, fs:fe]
            )
            if gi >= 2:
                # keep the gather-gated d2 chain BEHIND the previous group's
                # main DVE ops in the scheduled stream (the scheduler's DMA
                # model thinks indirect gathers are cheap; at runtime they'd
                # stall the whole in-order DVE stream if hoisted early)
                a = anchor_for(ts_ - 3)
                if a is not None:
                    add_dep_helper(sub_i.ins, a.ins, sync=False,
                                   reason="hold d2 chain behind prior group")
            elif gi == 1:
                a = anchor_for(1)
                if a is not None:
                    add_dep_helper(sub_i.ins, a.ins, sync=False,
                                   reason="hold d2 chain behind prior group")
            else:
                # group 0 only needs to sit behind an EARLY chat op, not the
                # whole build: chunk 1's csq keeps the DVE stream clean while
                # letting the first d2 complete ~5us sooner
                add_dep_helper(sub_i.ins, c2cp_early.ins, sync=False,
                               reason="hold g0 d2 chain behind chat chunk 1")
            with nc.allow_low_precision(reason="fp16 d2: d err ~1e-2 abs, random per row"):
                nc.vector.tensor_reduce(
                    d2[:, ts_:te], diff[:, fs:fe].rearrange("p (t d) -> p t d", d=D),
                    axis=AX.X, op=OP.add,
                )
            nc.scalar.activation(dall[:, ts_:te], d2[:, ts_:te], AF.Sqrt)
            if nD:
                nc.vector.tensor_scalar_mul(e2s[:, ts_:te], e2[:, ts_:te], 1.0 / SCL)
                nc.vector.tensor_scalar_mul(d2s[:, ts_:te], d2[:, ts_:te], 1.0 / SCL)

            # main tiles of this group
            for t in range(ts_, te):
                ps = mm_pool.tile([P, C], F32, name="ps")
                lhsT = eTa[:, t * P:(t + 1) * P]
                for k in range(NB):
                    nc.tensor.matmul(
                        ps[:, k * FD:(k + 1) * FD],
                        lhsT=lhsT,
                        rhs=chat[:, k * FD:(k + 1) * FD],
                        start=True, stop=True,
                    )
                ty = types[t]
                if ty == 2:
                    # D-tile: DVE evacuates PSUM, quadratic sqrt approx
                    m = dist_pool.tile([P, C], F16, name="dist")
                    mi = nc.vector.tensor_scalar(
                        out=m[:], in0=ps[:],
                        scalar1=e2s[:, t:t + 1], scalar2=d2s[:, t:t + 1],
                        op0=OP.add, op1=OP.min,
                    )
                    u = dist_pool.tile([P, C], F16, name="dist")
                    nc.vector.tensor_scalar(
                        out=u[:], in0=m[:], scalar1=PA2, scalar2=PA1,
                        op0=OP.mult, op1=OP.add,
                    )
                    nc.vector.tensor_mul(u[:], u[:], m[:])
                    nc.vector.tensor_add(acc[:], acc[:], u[:])
                    dve_anchor[t] = mi
                else:
                    dist = dist_pool.tile([P, C], F16, name="dist")
                    nc.scalar.activation(
                        dist[:], ps[:], AF.Sqrt,
                        bias=e2[:, t:t + 1], scale=SCL,
                    )
                    if ty == 1:
                        # VR-tile: the whole min+reduce runs on ACT:
                        # sum_j relu(d_i - d_ij) via scale=-1, bias=d_i
                        nc.scalar.activation(
                            dist[:], dist[:], AF.Relu,
                            bias=dall[:, t:t + 1], scale=-1.0,
                            accum_out=maccA[:, t:t + 1],
                        )
                    else:
                        mi = nc.vector.tensor_scalar(
                            out=dist[:], in0=dist[:],
                            scalar1=dall[:, t:t + 1], scalar2=None,
                            op0=OP.min,
                        )
                        nc.vector.tensor_add(acc[:], acc[:], dist[:])
                        dve_anchor[t] = mi

        mm_ctx.__exit__(None, None, None)

        # loss = C*(sum_i d_i - sum_{VR} d_i) - sum(acc) + sum(maccA)
        #        - a0*2048*128*nD
        nc.vector.tensor_reduce(rowacc[:], acc[:], axis=AX.X, op=OP.add)
        nc.vector.tensor_reduce(dsum[:], dall[:], axis=AX.X, op=OP.add)
        if n_va:
            nc.vector.tensor_reduce(rowA[:], maccA[:], axis=AX.X, op=OP.add)
        if n_va:
            stride = T // n_va
            dallv = dall[:].rearrange("p (g s) -> p s g", s=stride)
            nc.vector.tensor_reduce(
                dsumv[:], dallv[:, stride // 2:stride // 2 + 1, :],
                axis=AX.X, op=OP.add,
            )
            nc.vector.tensor_sub(dsum[:], dsum[:], dsumv[:])
        nc.vector.scalar_tensor_tensor(
            out=rowfin[:], in0=dsum[:], scalar=float(C), op0=OP.mult,
            in1=rowacc[:], op1=OP.subtract,
        )
        if n_va:
            nc.vector.tensor_add(rowfin[:], rowfin[:], rowA[:])
        with tc.tile_pool(name="fin", bufs=1, space="PSUM") as finp:
            fin = finp.tile([1, 1], F32)
            nc.tensor.matmul(fin[:], lhsT=rowfin[:], rhs=onesr[:], start=True, stop=True)
            nc.scalar.activation(
                outsb[:], fin[:], AF.Identity,
                bias=-PA0 * float(C) * float(P) * float(nD),
            )
        nc.sync.dma_start(out, outsb[:])


_NC_CACHE = {}


def build_nc(n_va=N_VA, n_d=N_D):
    key = (n_va, n_d)
    if key in _NC_CACHE:
        return _NC_CACHE[key]
    nc = bacc.Bacc(
        "TRN2", target_bir_lowering=False, debug=False, enable_asserts=False
    )
    eT = nc.dram_tensor("eT", [KA, NS], MM_DT, kind="ExternalInput").ap()
    enat = nc.dram_tensor("enat", [NS, D], F16, kind="ExternalInput").ap()
    labT = nc.dram_tensor("labT", [P, T], I32, kind="ExternalInput").ap()
    cT = nc.dram_tensor("cT", [D, C], F32, kind="ExternalInput").ap()
    cnat = nc.dram_tensor("cnat", [C, D], F32, kind="ExternalInput").ap()
    out = nc.dram_tensor("out", [1, 1], F32, kind="ExternalOutput").ap()
    with tile.TileContext(nc) as tc:
        _body(tc, out, eT, enat, labT, cT, cnat, n_va=n_va, n_d=n_d)
    nc.compile()
    _NC_CACHE[key] = nc
    return nc


def make_in_maps(embeddings, centers, labels):
    e = np.ascontiguousarray(np.asarray(embeddings, dtype=np.float32))
    c = np.ascontiguousarray(np.asarray(centers, dtype=np.float32))
    lab = np.asarray(labels).astype(np.int32)
    assert e.shape == (N, D) and c.shape == (C, D) and lab.shape == (N,)
    cT = np.ascontiguousarray(c.T)
    in_maps = []
    for core in range(NCORES):
        es = e[core * NS:(core + 1) * NS]
        ls = lab[core * NS:(core + 1) * NS]
        eTa = np.ones((KA, NS), np.float32)
        eTa[0:D] = es.T
        eTa = eTa.astype(np.float16)
        in_maps.append({
            "eT": eTa,
            "enat": np.ascontiguousarray(es).astype(np.float16),
            "labT": np.ascontiguousarray(ls.reshape(T, P).T),
            "cT": cT,
            "cnat": c,
        })
    return in_maps


def run(embeddings, centers, labels, n_va=N_VA, n_d=N_D, **kw):
    nc = build_nc(n_va, n_d)
    in_maps = make_in_maps(embeddings, centers, labels)
    res = run_bass_kernel_spmd(nc, in_maps, core_ids=list(range(NCORES)), **kw)
    total = float(sum(float(r["out"][0, 0]) for r in res.results))
    return np.float32(total), res


def kernel(embeddings, centers, labels):
    val, _ = run(embeddings, centers, labels)
    return val
